# revision 6
# baseline (speedup 1.0000x reference)
"""BoneLengthLoss Trainium2 kernel.

Full inputs: pose_3d_pred (524288, 37, 3) f32, pose_3d_ref same, valid_mask
(524288, 37) bool.  Output: scalar f32 = sum(sq_err * bone_valid) /
sum(bone_valid) over all (batch, bone) pairs.

Strategy: pure data-parallel over 8 NeuronCores (batch dim).  Per core:
65536 batch rows, processed in T tiles of 128 partitions x R rows each
(one batch row = 111 f32 = 37 kpts x 3).  The static bone gather
(J1/J2 endpoint indexing) is decomposed into 13 arithmetic "runs" so each
gathered subtraction is a single strided-AP vector op instead of 32
per-bone ops.  DVE does the 2-source ops (gathered diffs, triple sums,
length diff, mask AND, masked-square-accumulate via tensor_tensor_reduce);
ACT does squares and sqrt.  Each core returns per-partition partial
(num, den); the host sums 8x128 partials and divides.
"""

import sys

sys.path.insert(0, "/opt/trn_rl_repo")

import numpy as np

# ---- problem constants (hardcoded; kernel.py must be self-contained) ----
N_CORES = 8
BATCH = 524288
KP = 37  # keypoints
NB = 32  # bones
B_CORE = BATCH // N_CORES  # 65536
P = 128  # SBUF partitions
R = 32  # batch rows per partition per tile
T = B_CORE // (P * R)  # tiles per core
RB = R * NB  # bone entries per partition per tile
ROW = KP * 3  # 111 floats per batch row

# Bone list decomposed into runs: (j1_start, s1, j2_start, s2, L).
# Bone i of a run connects joints (j1_start + i*s1, j2_start + i*s2).
# Output bone order = concatenation of runs (a permutation of the
# reference bone order — irrelevant, everything is summed).
RUNS = [
    (1, 0, 2, 1, 3),
    (2, 1, 5, 1, 2),
    (11, 0, 12, 1, 2),
    (12, 1, 14, 0, 2),
    (14, 1, 15, 1, 3),
    (12, 1, 18, 1, 2),
    (18, 1, 20, 1, 4),
    (16, 0, 24, 1, 2),
    (24, 1, 26, 0, 2),
    (24, 1, 27, 1, 2),
    (27, 1, 29, 1, 4),
    (17, 1, 33, 16, 1),
    (33, 1, 34, 1, 3),
]
assert sum(r[4] for r in RUNS) == NB

_COMPILED = None


def _build(T=T):
    from concourse import bacc, tile
    import concourse.mybir as mybir

    f32 = mybir.dt.float32
    u8 = mybir.dt.uint8
    DT = f32  # intermediate dtype

    nc = bacc.Bacc("TRN2", target_bir_lowering=False, debug=False)

    pred_d = nc.dram_tensor("pred", [T, P, R * ROW], f32, kind="ExternalInput")
    ref_d = nc.dram_tensor("ref", [T, P, R * ROW], f32, kind="ExternalInput")
    mask_d = nc.dram_tensor("mask", [T, P, R * KP], u8, kind="ExternalInput")
    out_d = nc.dram_tensor("out", [P, 2], f32, kind="ExternalOutput")

    with tile.TileContext(nc) as tc:
        with (
            tc.tile_pool(name="io", bufs=2) as io_pool,
            tc.tile_pool(name="work", bufs=2) as work_pool,
            tc.tile_pool(name="acc", bufs=1) as acc_pool,
        ):
            numstrip = acc_pool.tile([P, T], f32)
            denstrip = acc_pool.tile([P, T], f32)

            for t in range(T):
                pq = io_pool.tile([P, 2 * R * ROW], f32)
                m8 = io_pool.tile([P, R * KP], u8)
                nc.gpsimd.dma_start(pq[:, : R * ROW], pred_d[t])
                nc.gpsimd.dma_start(pq[:, R * ROW :], ref_d[t])
                nc.gpsimd.dma_start(m8[:], mask_d[t])

                # D: bone-difference planes, layout [p][c(3)][pose(2)][r][b(32)]
                D = work_pool.tile([P, 6 * RB], DT)
                pqv = pq.rearrange("p (g r k c) -> p g r k c", g=2, r=R, k=KP, c=3)
                Dv = D.rearrange("p (c g r b) -> p c g r b", c=3, g=2, r=R, b=NB)
                for g in range(2):
                    pos = 0
                    for j1, s1, j2, s2, L in RUNS:
                        if s1 == 0:
                            a1 = pqv[:, g, :, j1 : j1 + 1, :].to_broadcast(
                                [P, R, L, 3]
                            )
                        else:
                            a1 = pqv[:, g, :, j1 : j1 + s1 * (L - 1) + 1 : s1, :]
                        if s2 == 0:
                            a2 = pqv[:, g, :, j2 : j2 + 1, :].to_broadcast(
                                [P, R, L, 3]
                            )
                        else:
                            a2 = pqv[:, g, :, j2 : j2 + s2 * (L - 1) + 1 : s2, :]
                        # out slice [p, c, r, L] -> transpose to [p, r, L, c]
                        o = Dv[:, :, g, :, pos : pos + L].transpose([0, 2, 3, 1])
                        # pred_vec = pose[J2] - pose[J1]
                        nc.vector.tensor_sub(o, a2, a1)
                        pos += L

                # squares in place (ACT), then triple-sum -> squared lengths
                nc.scalar.square(D[:], D[:])
                L2 = work_pool.tile([P, 2 * RB], DT)
                nc.vector.tensor_add(L2[:], D[:, : 2 * RB], D[:, 2 * RB : 4 * RB])
                nc.vector.tensor_add(L2[:], L2[:], D[:, 4 * RB :])
                # lengths (in-place sqrt)
                nc.scalar.sqrt(L2[:], L2[:])

                # E = pred_len - ref_len
                E = work_pool.tile([P, RB], DT)
                nc.vector.tensor_sub(E[:], L2[:, :RB], L2[:, RB:])

                # bone_valid = mask[J1] * mask[J2]  (u8 -> DT)
                MV = work_pool.tile([P, RB], DT)
                m8v = m8.rearrange("p (r k) -> p r k", r=R, k=KP)
                MVv = MV.rearrange("p (r b) -> p r b", r=R, b=NB)
                pos = 0
                for j1, s1, j2, s2, L in RUNS:
                    if s1 == 0:
                        b1 = m8v[:, :, j1 : j1 + 1].to_broadcast([P, R, L])
                    else:
                        b1 = m8v[:, :, j1 : j1 + s1 * (L - 1) + 1 : s1]
                    if s2 == 0:
                        b2 = m8v[:, :, j2 : j2 + 1].to_broadcast([P, R, L])
                    else:
                        b2 = m8v[:, :, j2 : j2 + s2 * (L - 1) + 1 : s2]
                    nc.vector.tensor_tensor(
                        MVv[:, :, pos : pos + L], b1, b2, op=mybir.AluOpType.mult
                    )
                    pos += L

                # masked error; then ACT does square+row-sum (num) and
                # copy+row-sum (den) via activation accumulate
                ME = work_pool.tile([P, RB], DT)
                nc.vector.tensor_tensor(ME[:], E[:], MV[:], op=mybir.AluOpType.mult)
                nc.scalar.activation(
                    ME[:],
                    ME[:],
                    mybir.ActivationFunctionType.Square,
                    accum_out=numstrip[:, t : t + 1],
                )
                nc.scalar.activation(
                    MV[:],
                    MV[:],
                    mybir.ActivationFunctionType.Copy,
                    accum_out=denstrip[:, t : t + 1],
                )

            acc2 = acc_pool.tile([P, 2], f32)
            nc.vector.reduce_sum(acc2[:, 0:1], numstrip[:], axis=mybir.AxisListType.X)
            nc.vector.reduce_sum(acc2[:, 1:2], denstrip[:], axis=mybir.AxisListType.X)
            nc.gpsimd.dma_start(out_d[:], acc2[:])

    nc.compile()
    return nc


def _get_nc():
    global _COMPILED
    if _COMPILED is None:
        _COMPILED = _build()
    return _COMPILED


def _make_in_maps(pose_3d_pred, pose_3d_ref, valid_mask):
    pred = np.ascontiguousarray(np.asarray(pose_3d_pred, dtype=np.float32))
    ref = np.ascontiguousarray(np.asarray(pose_3d_ref, dtype=np.float32))
    mask = np.ascontiguousarray(np.asarray(valid_mask)).astype(np.uint8)
    in_maps = []
    for c in range(N_CORES):
        sl = slice(c * B_CORE, (c + 1) * B_CORE)
        in_maps.append(
            {
                "pred": pred[sl].reshape(T, P, R * ROW),
                "ref": ref[sl].reshape(T, P, R * ROW),
                "mask": mask[sl].reshape(T, P, R * KP),
            }
        )
    return in_maps


def kernel(pose_3d_pred, pose_3d_ref, valid_mask, _trace=False):
    from concourse.bass_utils import run_bass_kernel_spmd

    nc = _get_nc()
    in_maps = _make_in_maps(pose_3d_pred, pose_3d_ref, valid_mask)
    res = run_bass_kernel_spmd(nc, in_maps, list(range(N_CORES)), trace=_trace)
    num = 0.0
    den = 0.0
    for i in range(N_CORES):
        o = res.results[i]["out"].astype(np.float64)
        num += o[:, 0].sum()
        den += o[:, 1].sum()
    out = np.float32(num / den)
    if _trace:
        return out, res
    return out
